# revision 10
# baseline (speedup 1.0000x reference)
"""GCN layer on 8 Trainium2 NeuronCores — v5.

Computes relu(D^-1/2 (A+I) D^-1/2 X W + b) for N=8192, d=256.

vs baseline (197us):
- adj stored/streamed/matmul'd as fp8-e3m4 (8MB/core; mixed bf16 x e3m4
  PE matmul measured exact vs the e3m4-quantized reference).
- The 42us degree AllGather collective is replaced by 7 XOR remote-DMA
  broadcasts of each core's local D^-1/2 block, one per peer slot, each on
  its own remote semaphore; consumers are gated PER SLOT so the main matmul
  overlaps the exchange (group g starts when slot g lands).
- j-blocks are XOR-permuted per core on the host (slot g of core c holds
  block c^g, with bit 1 flipped for cross-die g>=4 — measured D2D routing).
- x streams behind adj on the sync DGE ring (adj stream owns HBM first);
  W/b/eye + a tiny barrier input go on the scalar ring.
- A PE warm-up burst trips HAM to 2.4GHz before rowsums chase the stream.
- A dummy background AllGather keeps NRT on the aligned multi-rank launch
  path (without it, core launches skew by ~1ms each and the profiled core
  idles in a 5+ms semaphore wait); a terminal gpsimd wait keeps teardown
  from racing the collective.
"""

import numpy as np

N = 8192
D = 256
NCORES = 8
R = N // NCORES      # rows per core = 1024
KT = N // 128        # 64 j-tiles
G = NCORES           # 8 j-groups (one per peer)
TPG = KT // G        # 8 tiles per group
TS = R // 128        # 8 own-row tiles

_CACHE = {}


def _build_nc():
    import concourse.bacc as bacc
    import concourse.tile as tile
    import concourse.mybir as mybir

    f32 = mybir.dt.float32
    bf16 = mybir.dt.bfloat16
    f8e3 = mybir.dt.float8e3
    AF = mybir.ActivationFunctionType

    nc = bacc.Bacc("TRN2", target_bir_lowering=False, debug=False,
                   num_devices=NCORES)

    adjS = nc.dram_tensor("adjS", [128, KT * R], f8e3, kind="ExternalInput")
    xS = nc.dram_tensor("xS", [128, KT * D], bf16, kind="ExternalInput")
    Win = nc.dram_tensor("W", [D, D], bf16, kind="ExternalInput")
    bin_ = nc.dram_tensor("b", [D], f32, kind="ExternalInput")
    eyeb = nc.dram_tensor("eye", [128, 128], bf16, kind="ExternalInput")
    eyef = nc.dram_tensor("eyef", [128, 128], f32, kind="ExternalInput")
    outT = nc.dram_tensor("outT", [D, R], f32, kind="ExternalOutput")

    # per-(engine, group) first-consumer names for the injected slot waits
    sc_names = {("vector", g): [] for g in range(1, G)}
    sc_names.update({("scalar", g): [] for g in range(1, G)})
    box = {}

    with tile.TileContext(nc) as tc:
        from contextlib import ExitStack

        with ExitStack() as ctx:
            pp = ctx.enter_context(tc.tile_pool(name="persist", bufs=1))
            dp = ctx.enter_context(tc.tile_pool(name="dram", bufs=1, space="DRAM"))

            # ---- persistent SBUF tensors ----
            adjTb = pp.tile([128, KT * R], f8e3)   # 64KB/partition cache
            xb = pp.tile([128, KT * D], bf16)      # x, partition = j%128
            Wb = pp.tile([128, 2 * D], bf16)       # W, partition = k%128
            bsb = pp.tile([128, 2], f32)           # bias, partition = m%128
            eye_s = pp.tile([128, 128], bf16)
            eyef_s = pp.tile([128, 128], f32)
            ones8 = pp.tile([128, 1], f8e3)        # rowsum lhsT
            warm8 = pp.tile([128, 512], f8e3)      # PE warm-up rhs
            deg_s = pp.tile([1, R], f32)           # local degree (+1)
            degln = pp.tile([8, 128], f32)         # local degrees, natural
            disal = pp.tile([128, G * TPG], f32)   # dis table, slot g cols 8g..
            disl = pp.tile([1, R], f32)            # local D^-1/2 (free layout)
            disrep = pp.tile([128, R], f32)        # own D^-1/2 on free dim
            y2 = [pp.tile([128, R], bf16, name=f"y2_{i}") for i in range(2)]
            outsb = [pp.tile([128, R], f32, name=f"outsb_{i}") for i in range(2)]
            barsb = pp.tile([1, 8], f32)

            disl_d = dp.tile([R], f32)
            degl_d = dp.tile([R], f32)
            bar_in = dp.tile([8], f32)
            bar_out = dp.tile([8 * NCORES], f32)

            nc.any.memset(ones8[:], 1.0)
            nc.gpsimd.memset(warm8[:], 0.0)
            nc.gpsimd.memset(barsb[:], 0.0)

            # ---- phase 1: streams ----
            # scalar ring: barrier input + x group 0 + W/b/eyes.
            nc.scalar.dma_start(out=bar_in[:], in_=barsb[0:1, :])
            nc.scalar.dma_start(out=xb[:, 0:TPG * D], in_=xS.ap()[:, 0:TPG * D])
            nc.scalar.dma_start(
                out=Wb[:, :].rearrange("p (k m) -> p k m", m=D),
                in_=Win.ap().rearrange("(k p) m -> p k m", p=128))
            nc.scalar.dma_start(
                out=bsb[:, :], in_=bin_.ap().rearrange("(h p) -> p h", p=128))
            nc.scalar.dma_start(out=eye_s[:, :], in_=eyeb.ap())
            nc.scalar.dma_start(out=eyef_s[:, :], in_=eyef.ap())

            # sync ring: adj slices sequential (FIFO per ring) so rowsums
            # chase the stream; final slices finer for a tight degree tail;
            # x groups 1..7 behind adj (consumed at matmul pace).
            SL = [(g * TPG, TPG) for g in range(G - 1)]
            SL += [(56, 4), (60, 2), (62, 1), (63, 1)]
            for t0, nt in SL:
                c0, c1 = t0 * R, (t0 + nt) * R
                nc.sync.dma_start(out=adjTb[:, c0:c1], in_=adjS.ap()[:, c0:c1])
            for g in range(1, G):
                c0, c1 = g * TPG * D, (g + 1) * TPG * D
                nc.sync.dma_start(out=xb[:, c0:c1], in_=xS.ap()[:, c0:c1])

            pdeg = ctx.enter_context(
                tc.tile_pool(name="psdeg", bufs=1, space="PSUM"))
            pst = ctx.enter_context(
                tc.tile_pool(name="pst", bufs=1, space="PSUM"))
            psuo = ctx.enter_context(
                tc.tile_pool(name="psuo", bufs=2, space="PSUM"))

            # PE warm-up burst: back-to-back N=512 pumps trip HAM to 2.4GHz
            # before the first adj group lands.
            wps = pst.tile([1, 512], f32, padded_shape=[128, 512], name="wps")
            for _ in range(14):
                nc.tensor.matmul(wps[:, :], ones8[:, :], warm8[:, :],
                                 start=True, stop=True,
                                 skip_group_check=True)

            # rowsums (pure fp8 matmuls) chase the stream
            dps = pdeg.tile([1, 1024], f32, padded_shape=[128, 1024])
            for k in range(KT):
                for s in range(2):
                    nc.tensor.matmul(
                        dps[:, s * 512:(s + 1) * 512], ones8[:, :],
                        adjTb[:, k * R + s * 512:k * R + (s + 1) * 512],
                        start=(k == 0), stop=(k == KT - 1),
                        skip_group_check=True)
            # deg = rowsum + 1 (the +I term)
            for s in range(2):
                nc.vector.tensor_scalar_add(
                    deg_s[:, s * 512:(s + 1) * 512],
                    dps[:, s * 512:(s + 1) * 512], 1.0)

            # ---- phase 2: own dis block + P2P exchange ----
            # deg [1,1024] -> DRAM -> [8,128] natural -> PE transpose [128,8]
            tps = pst.tile([128, TPG], f32)
            nc.scalar.dma_start(out=degl_d[:], in_=deg_s[0:1, :])
            nc.scalar.dma_start(
                out=degln[:, :],
                in_=degl_d.opt().rearrange("(c f) -> c f", f=128))
            nc.tensor.transpose(tps[:, 0:TPG], degln[:, :],
                                eyef_s[0:TPG, 0:TPG])
            nc.vector.reciprocal_approx_fast(disal[:, 0:TPG], tps[:, 0:TPG])
            nc.scalar.activation(disal[:, 0:TPG], disal[:, 0:TPG], AF.Sqrt)

            # prepared remote broadcasts: slot k -> XOR peer (own ^ k), one
            # remote sem per slot so consumers gate on their own slot.
            rsems = [None] + [nc.alloc_semaphore(f"dis_rsem{k}")
                              for k in range(1, NCORES)]
            lsem = nc.alloc_semaphore("dis_lsem")
            box["rsems"] = rsems
            for k in range(1, NCORES):
                rdests = [None] * 8
                rdests[k] = (0, k)
                nc.gpsimd.remote_dma_broadcast(
                    out_ap=disal[:, TPG * k:TPG * (k + 1)],
                    in_ap=disal[:, 0:TPG],
                    remote_sem=rsems[k], local_sem=lsem, rdests=rdests)
            nc.gpsimd.trigger_dma(count=None)

            # dummy background AllGather for aligned multi-rank launch;
            # posted right after the trigger on the gpsimd stream.
            nc.gpsimd.collective_compute(
                "AllGather", mybir.AluOpType.bypass,
                replica_groups=[list(range(NCORES))],
                ins=[bar_in.opt()], outs=[bar_out.opt()])

            # local dis for the output-side (free-dim) scaling, via DRAM
            # roundtrip on the scalar ring; off the critical path.
            nc.vector.reciprocal_approx_fast(disl[:, :], deg_s[:, :])
            nc.scalar.activation(disl[:, :], disl[:, :], AF.Sqrt)
            nc.scalar.dma_start(out=disl_d[:], in_=disl[0:1, :])
            nc.scalar.dma_start(
                out=disrep[:, :],
                in_=disl_d.opt().unsqueeze(0).partition_broadcast(128))

            # ---- phase 3: y = dis*x, then U^T = ((A+I) y)^T ----
            u = [psuo.tile([128, R], f32, name=f"u_{i}", tag="uo")
                 for i in range(2)]

            def scale_y(k):
                chunk = xb[:, k * D:(k + 1) * D]
                g = k // TPG
                if k % 2 == 0:
                    i = nc.scalar.activation(chunk, chunk, AF.Copy,
                                             scale=disal[:, k:k + 1])
                    if g >= 1:
                        sc_names[("scalar", g)].append(i.ins.name)
                else:
                    i = nc.vector.tensor_scalar_mul(chunk, chunk,
                                                    disal[:, k:k + 1])
                    if g >= 1:
                        sc_names[("vector", g)].append(i.ins.name)

            def mm_group(g, start, stop=False):
                for kk in range(TPG):
                    k = g * TPG + kk
                    for h in range(2):
                        for s in range(2):
                            nc.tensor.matmul(
                                u[h][:, s * 512:(s + 1) * 512],
                                xb[:, k * D + h * 128:k * D + (h + 1) * 128],
                                adjTb[:, k * R + s * 512:k * R + (s + 1) * 512],
                                start=start and kk == 0,
                                stop=stop and kk == TPG - 1,
                                skip_group_check=True)

            # group 0 (own block) needs only local dis; runs while the
            # exchange is in flight. +I follows immediately (its deps are
            # ready early); the stop rides on the true-last group-7 matmul.
            for k in range(TPG):
                scale_y(k)
            mm_group(0, start=True)
            for t in range(TS):
                for h in range(2):
                    nc.tensor.matmul(
                        u[h][:, t * 128:(t + 1) * 128],
                        xb[:, t * D + h * 128:t * D + (h + 1) * 128],
                        eye_s[:, :],
                        start=False, stop=False,
                        skip_group_check=True)

            # remaining groups: scale + matmul per group, slot-gated (waits
            # injected post-scheduling)
            for g in range(1, G):
                for k in range(g * TPG, (g + 1) * TPG):
                    scale_y(k)
                mm_group(g, start=False, stop=(g == G - 1))

            # ---- phase 4: scale columns by own dis ----
            for h in range(2):
                nc.vector.tensor_mul(y2[h][:, :], u[h][:, :], disrep[:, :])

            # ---- phase 5: out^T = W^T @ (scaled U^T) ----
            o = [psuo.tile([128, R], f32, name=f"o_{i}", tag="uo")
                 for i in range(2)]
            for mh in range(2):
                for nk in range(2):
                    for s in range(2):
                        nc.tensor.matmul(
                            o[mh][:, s * 512:(s + 1) * 512],
                            Wb[:, nk * D + mh * 128:nk * D + (mh + 1) * 128],
                            y2[nk][:, s * 512:(s + 1) * 512],
                            start=(nk == 0), stop=(nk == 1),
                            skip_group_check=True)

            # ---- phase 6: relu(out^T + b), write transposed output ----
            for mh in range(2):
                nc.scalar.activation(
                    outsb[mh][:, :], o[mh][:, :], AF.Relu,
                    bias=bsb[:, mh:mh + 1], scale=1.0)
                nc.sync.dma_start(
                    out=outT.ap()[mh * 128:(mh + 1) * 128, :],
                    in_=outsb[mh][:, :])

            # teardown must not race the background collective: a tiny DMA
            # reading its output picks up a RAW dep on the collective, so
            # the NEFF end waits for it.
            barck = pp.tile([1, 8 * NCORES], f32)
            nc.scalar.dma_start(out=barck[:, :],
                                in_=bar_out.opt().unsqueeze(0))

    # Inject the per-slot remote-arrival waits post-scheduling (the tile
    # scheduler's single-core sim cannot model peer sem increments and
    # would deadlock on an in-tile wait). For each engine and group, a
    # wait on that slot's sem goes right before the group's first
    # consumer in the scheduled stream.
    rsems = box["rsems"]
    for eng in ("vector", "scalar"):
        for g in range(1, G):
            nameset = set(sc_names[(eng, g)])
            if not nameset:
                continue
            w = getattr(nc, eng).wait_ge(rsems[g], 2)
            for b in nc.main_func.blocks:
                if w.ins in b.instructions:
                    b.instructions.remove(w.ins)
            tgt = None
            for b in nc.main_func.blocks:
                hits = [i for i, ins in enumerate(b.instructions)
                        if ins.name in nameset]
                if hits:
                    tgt = (b, min(hits))
                    break
            assert tgt is not None, f"no {eng}/{g} remote consumers found"
            tgt[0].instructions.insert(tgt[1], w.ins)

    nc.compile()
    return nc


def _get_nc():
    if "nc" not in _CACHE:
        _CACHE["nc"] = _build_nc()
    return _CACHE["nc"]


def _sbuf_image(mat):
    """[T*128, F] -> [128, T*F] where partition p holds rows {128t+p}."""
    t128, f = mat.shape
    t = t128 // 128
    return np.ascontiguousarray(
        mat.reshape(t, 128, f).transpose(1, 0, 2).reshape(128, t * f))


def kernel(x, adj, W, b):
    import ml_dtypes
    from concourse.bass_utils import run_bass_kernel_spmd

    bf = ml_dtypes.bfloat16
    f8 = ml_dtypes.float8_e3m4
    x = np.asarray(x, dtype=np.float32)
    adj = np.asarray(adj, dtype=np.float32)
    W = np.ascontiguousarray(np.asarray(W, dtype=np.float32)).astype(bf)
    b = np.ascontiguousarray(np.asarray(b, dtype=np.float32))

    nc = _get_nc()

    x_bf = np.ascontiguousarray(x).astype(bf)
    adj8 = adj.astype(f8)
    eye_np = np.eye(128, dtype=bf)
    eyef_np = np.eye(128, dtype=np.float32)
    in_maps = []
    for c in range(NCORES):
        rows_c = slice(c * R, (c + 1) * R)
        adj_groups = []
        x_groups = []
        for g in range(G):
            # remote slot g receives from XOR peer; the cross-die hop
            # (bit 2) lands on the D2D-diagonal SEngine, flipping bit 1
            # (measured routing).
            blk = c ^ g ^ (2 if g >= 4 else 0)
            rows_b = slice(blk * R, (blk + 1) * R)
            adj_groups.append(
                _sbuf_image(np.ascontiguousarray(adj8[rows_c, rows_b].T)))
            x_groups.append(_sbuf_image(x_bf[rows_b, :]))
        in_maps.append({
            "adjS": np.ascontiguousarray(np.concatenate(adj_groups, axis=1)),
            "xS": np.ascontiguousarray(np.concatenate(x_groups, axis=1)),
            "W": W,
            "b": b,
            "eye": eye_np,
            "eyef": eyef_np,
        })

    res = run_bass_kernel_spmd(nc, in_maps, core_ids=list(range(NCORES)))
    _CACHE["last_res"] = res
    out = np.concatenate(
        [np.asarray(res.results[c]["outT"]).T for c in range(NCORES)], axis=0)
    return np.ascontiguousarray(out, dtype=np.float32)


if __name__ == "__main__":
    rng = np.random.default_rng(0)
    x = rng.standard_normal((N, D)).astype(np.float32)
    adj = rng.random((N, N)).astype(np.float32)
    W = rng.standard_normal((D, D)).astype(np.float32) * 0.06
    b = rng.standard_normal((D, )).astype(np.float32) * 0.06
    out = kernel(x=x, adj=adj, W=W, b=b)
    print(out.shape, out.dtype)
